# revision 25
# baseline (speedup 1.0000x reference)
# Multi-head attention (B=2, S=2048, D=1024, H=16) on 8 TRN2 NeuronCores.
#
# Sharding (hardcoded): core c in [0..8) handles batch b = c//4 and head
# group g = c%4 (4 heads = 256 output features of wq/wk/wv, 256 input rows
# of wo). Each core computes a partial output projection [S, D]; the host
# sums the 4 partials per batch and adds wo_bias (row-parallel unshard).
#
# Device-side layout choices:
#   - activations enter transposed ([D, S]) so every matmul contracts over
#     the partition axis with no on-device transposes;
#   - scores are computed transposed (S^T[k, q]) so softmax(P) feeds the
#     P@V matmul directly (contraction over k on partitions);
#   - the softmax denominator comes free as an extra ones-column appended
#     to each head's V block (output row 64 of the PV accumulation);
#   - matmuls run in float32r (full-rate fp32 path for moving dim >= 256);
#     P/V/out-proj run in bf16.
import functools
import sys

import numpy as np

try:
    import concourse  # noqa: F401
except ImportError:  # harness env without the default path
    sys.path.insert(0, "/opt/trn_rl_repo")
    sys.path.insert(0, "/opt/pypackages")

import ml_dtypes

BF16 = ml_dtypes.bfloat16

B, S, D, H = 2, 2048, 1024, 16
HD = D // H          # 64
NCORES = 8
GH = 4               # head groups (tensor-parallel)
HPG = H // GH        # heads per group = 4
DG = D // GH         # features per group = 256
P = 128              # partitions
TDIN = D // P        # 8 din tiles
SC = 4               # s-chunks of 512 for projections
CW = S // SC         # 512
QC = 2               # q-chunks of 1024 for attention
QW = S // QC         # 1024
KT = S // P          # 16 k tiles
NT2 = DG // P        # 2 dout tiles per group


def build_graph():
    """Build the SPMD Bass graph (identical on all 8 cores)."""
    from contextlib import ExitStack

    from concourse import bacc, mybir, tile

    f32 = mybir.dt.float32
    f32r = mybir.dt.float32r
    bf16 = mybir.dt.bfloat16
    EXP = mybir.ActivationFunctionType.Exp

    nc = bacc.Bacc(
        "TRN2", target_bir_lowering=False, debug=False, num_devices=NCORES
    )

    xq = nc.dram_tensor("xq_t", (D, S), bf16, kind="ExternalInput")
    xk = nc.dram_tensor("xk_t", (D, S), bf16, kind="ExternalInput")
    xv = nc.dram_tensor("xv_t", (D, S), bf16, kind="ExternalInput")
    mk = nc.dram_tensor("mask_t", (S, S), bf16, kind="ExternalInput")
    wq = nc.dram_tensor("wq", (D, DG), bf16, kind="ExternalInput")
    wk = nc.dram_tensor("wk", (D, DG), bf16, kind="ExternalInput")
    wv = nc.dram_tensor("wv", (D, DG), bf16, kind="ExternalInput")
    # wo pre-arranged host-side to [64, HPG, D] (j, h, n) so each head's
    # 64 rows sit on partitions 0..63.
    wo = nc.dram_tensor("wo", (HD, HPG, D), bf16, kind="ExternalInput")
    qb = nc.dram_tensor("qb", (1, DG), bf16, kind="ExternalInput")
    kb = nc.dram_tensor("kb", (1, DG), bf16, kind="ExternalInput")
    vb = nc.dram_tensor("vb", (1, DG), bf16, kind="ExternalInput")
    out = nc.dram_tensor("out", (S, D), bf16, kind="ExternalOutput")

    with tile.TileContext(nc) as tc, ExitStack() as ctx:
        wpool = ctx.enter_context(tc.tile_pool(name="wpool", bufs=1))
        cpool = ctx.enter_context(tc.tile_pool(name="cpool", bufs=1))
        qkpool = ctx.enter_context(tc.tile_pool(name="qk", bufs=1))
        vpool = ctx.enter_context(tc.tile_pool(name="vsb", bufs=1))
        mpool = ctx.enter_context(tc.tile_pool(name="msk", bufs=1))
        ppool = ctx.enter_context(tc.tile_pool(name="ptile", bufs=3))
        spool = ctx.enter_context(tc.tile_pool(name="small", bufs=2))
        dpool = ctx.enter_context(tc.tile_pool(name="dscr", bufs=2, space="DRAM"))
        bigps = ctx.enter_context(tc.tile_pool(name="bigps", bufs=3, space="PSUM"))
        ops_pool = ctx.enter_context(tc.tile_pool(name="ops", bufs=1, space="PSUM"))

        # ---- persistent SBUF tensors -------------------------------------
        wq_sb = wpool.tile([P, TDIN, DG], bf16)
        wk_sb = wpool.tile([P, TDIN, DG], bf16)
        wv_sb = wpool.tile([P, TDIN, DG], bf16)
        for wsb_, wdr_ in ((wq_sb, wq), (wk_sb, wk), (wv_sb, wv)):
            wr_ = wdr_.ap().rearrange("(t p) n -> p t n", p=P)
            for t_ in range(TDIN):
                nc.sync.dma_start(wsb_[:, t_, :], wr_[:, t_, :])
        wo_sb = wpool.tile([HD, HPG, D], bf16)
        nc.sync.dma_start(wo_sb[:], wo.ap())
        qb_sb = cpool.tile([1, DG], bf16)
        kb_sb = cpool.tile([1, DG], bf16)
        vb_sb = cpool.tile([1, DG], bf16)
        nc.sync.dma_start(qb_sb[:], qb.ap())
        nc.sync.dma_start(kb_sb[:], kb.ap())
        nc.sync.dma_start(vb_sb[:], vb.ap())
        # ones: row 0 used as [1, CW] rhs / [1, P] lhsT at partition 0;
        # row 64 used as [1, HD] lhsT at partition 64 (denominator bcast).
        ones2 = cpool.tile([1, CW], bf16)
        nc.vector.memset(ones2[:], 1.0)

        qT_sb = qkpool.tile([P, NT2, S], bf16)   # q projection, transposed
        kT_sb = qkpool.tile([P, NT2, S], bf16)
        # v blocks: per k-tile, per head: [v(64) | ones] -> 65 cols
        v_sb = vpool.tile([P, KT, HPG * (HD + 1)], bf16)
        nc.vector.memset(
            v_sb[:].rearrange("p s (h x) -> p s h x", h=HPG)[:, :, :, HD : HD + 1],
            1.0,
        )
        # ---- projections -------------------------------------------------
        # q, k: out qT[dout, s] = wq^T(stationary) x q^T(moving) + bias
        xpool_cm = tc.tile_pool(name="xin", bufs=3)
        xpool = xpool_cm.__enter__()
        for xdram, wsb, bias_sb, dest in (
            (xq, wq_sb, qb_sb, qT_sb),
            (xk, wk_sb, kb_sb, kT_sb),
        ):
            x_r = xdram.ap().rearrange("(t p) s -> p t s", p=P)
            for sc in range(SC):
                xch = xpool.tile([P, TDIN, CW], bf16, tag="xch")
                for half_ in range(2):
                    nc.sync.dma_start(
                        xch[:, half_ * 4 : (half_ + 1) * 4, :],
                        x_r[:, half_ * 4 : (half_ + 1) * 4, sc * CW : (sc + 1) * CW],
                    )
                for dt in range(NT2):
                    ps = bigps.tile([P, CW], f32, tag="ps")
                    for ktl in range(TDIN):
                        nc.tensor.matmul(
                            ps[:],
                            lhsT=wsb[:, ktl, dt * P : (dt + 1) * P],
                            rhs=xch[:, ktl, :],
                            start=(ktl == 0),
                            stop=False,
                        )
                    # bias: lhsT=[1,128] bias row, rhs=[1,CW] ones
                    nc.tensor.matmul(
                        ps[:],
                        lhsT=bias_sb[0:1, dt * P : (dt + 1) * P],
                        rhs=ones2[0:1, :],
                        start=False,
                        stop=True,
                    )
                    nc.scalar.copy(
                        dest[:, dt, sc * CW : (sc + 1) * CW], ps[:]
                    )
        # v: natural layout [s, dout] + bias, drained per-head with ones col
        xv_r = xv.ap().rearrange("(t p) s -> p t s", p=P)
        for sc in range(SC):
            xch = xpool.tile([P, TDIN, CW], bf16, tag="xch")
            for half_ in range(2):
                nc.sync.dma_start(
                    xch[:, half_ * 4 : (half_ + 1) * 4, :],
                    xv_r[:, half_ * 4 : (half_ + 1) * 4, sc * CW : (sc + 1) * CW],
                )
            for m in range(CW // P):
                st = sc * (CW // P) + m
                ps = bigps.tile([P, DG], f32, tag="ps")
                for ktl in range(TDIN):
                    nc.tensor.matmul(
                        ps[:],
                        lhsT=xch[:, ktl, m * P : (m + 1) * P],
                        rhs=wv_sb[:, ktl, :],
                        start=(ktl == 0),
                        stop=False,
                    )
                nc.tensor.matmul(
                    ps[:],
                    lhsT=ones2[0:1, 0:P],
                    rhs=vb_sb[:],
                    start=False,
                    stop=True,
                )
                nc.scalar.copy(
                    v_sb[:, st, :].rearrange("p (h x) -> p h x", h=HPG)[
                        :, :, 0:HD
                    ],
                    ps[:].rearrange("p (h x) -> p h x", h=HPG),
                )

        xpool_cm.__exit__(None, None, None)

        # mask load issued after projection DMAs so it doesn't hog queues
        mask_sb = mpool.tile([P, KT, S], bf16)
        mk_r = mk.ap().rearrange("(t p) q -> p t q", p=P)
        for kt in range(KT):
            nc.sync.dma_start(mask_sb[:, kt, :], mk_r[:, kt, :])

        # ---- attention ---------------------------------------------------
        # One head at a time; score psum triple-buffered so the PE can run
        # up to 3 k-tiles ahead of the exp/mask/PV chain.
        opool_sb = ctx.enter_context(tc.tile_pool(name="otn", bufs=1))
        otn_sb = opool_sb.tile([HD, HPG, S], bf16)
        for qc in range(QC):
            for h in range(HPG):
                t, po = h // 2, (h % 2) * HD
                o_ps = ops_pool.tile(
                    [HD + 1, QW], f32, tag="ops", name=f"ops_{qc}_{h}"
                )
                for kt in range(KT):
                    s_ps = bigps.tile(
                        [P, QW], f32, tag="ps", name=f"sps_{qc}_{h}_{kt}"
                    )
                    for hf in range(2):
                        nc.tensor.matmul(
                            s_ps[:, hf * 512 : (hf + 1) * 512],
                            lhsT=kT_sb[po : po + HD, t, kt * P : (kt + 1) * P],
                            rhs=qT_sb[
                                po : po + HD,
                                t,
                                qc * QW + hf * 512 : qc * QW + (hf + 1) * 512,
                            ],
                            start=True,
                            stop=True,
                        )
                    pt = ppool.tile(
                        [P, QW], bf16, tag="p", name=f"pt_{qc}_{h}_{kt}"
                    )
                    nc.scalar.activation(pt[:], s_ps[:], EXP, scale=0.125)
                    meng = nc.gpsimd if kt % 4 == 3 else nc.vector
                    meng.tensor_mul(
                        pt[:], pt[:], mask_sb[:, kt, qc * QW : (qc + 1) * QW]
                    )
                    for hf in range(2):
                        nc.tensor.matmul(
                            o_ps[:, hf * 512 : (hf + 1) * 512],
                            lhsT=v_sb[:, kt, h * 65 : (h + 1) * 65],
                            rhs=pt[:, hf * 512 : (hf + 1) * 512],
                            start=(kt == 0),
                            stop=(kt == KT - 1),
                        )
                # softmax normalization (no PE): approx-recip of the
                # denominator row, DRAM-bounce broadcast, one TT multiply.
                rec65 = spool.tile([HD + 1, QW], f32, tag="rec")
                nc.vector.reciprocal_approx_fast(out=rec65[:], in_=o_ps[:])
                osb = spool.tile([HD, QW], f32, tag="osb")
                nc.vector.tensor_copy(osb[:], o_ps[0:HD, :])
                scr = dpool.tile([1, QW], f32, tag="scr", name=f"scr_{qc}_{h}")
                nc.sync.dma_start(scr[:], rec65[HD : HD + 1, :])
                rb = spool.tile([HD, QW], f32, tag="rb")
                nc.sync.dma_start(rb[:], scr[:].to_broadcast((HD, QW)))
                nc.vector.tensor_mul(
                    otn_sb[:, h, qc * QW : (qc + 1) * QW], osb[:], rb[:]
                )

            # ---- output projection for this q-chunk (overlaps next chunk's
            # attention on the PE) --------------------------------------
            for st in range(qc * (QW // P), (qc + 1) * (QW // P)):
                osb2 = ppool.tile(
                    [P, D], bf16, tag="outsb", name=f"outsb_{st}"
                )
                for nch in range(2):
                    op_ps = bigps.tile(
                        [P, 512], f32, tag="ps", name=f"ops2_{st}_{nch}"
                    )
                    for h in range(HPG):
                        nc.tensor.matmul(
                            op_ps[:],
                            lhsT=otn_sb[:, h, st * P : (st + 1) * P],
                            rhs=wo_sb[:, h, nch * 512 : (nch + 1) * 512],
                            start=(h == 0),
                            stop=(h == HPG - 1),
                        )
                    nc.vector.tensor_copy(
                        osb2[:, nch * 512 : (nch + 1) * 512], op_ps[:]
                    )
                nc.sync.dma_start(out.ap()[st * P : (st + 1) * P, :], osb2[:])

    nc.compile()
    return nc


@functools.lru_cache(maxsize=1)
def _graph():
    return build_graph()


def make_in_maps(
    query, key, value, mask,
    wq_kernel, wq_bias, wk_kernel, wk_bias,
    wv_kernel, wv_bias, wo_kernel, wo_bias,
):
    q = np.asarray(query, np.float32)
    k = np.asarray(key, np.float32)
    v = np.asarray(value, np.float32)
    mask = np.asarray(mask)
    wqk = np.asarray(wq_kernel, np.float32)
    wkk = np.asarray(wk_kernel, np.float32)
    wvk = np.asarray(wv_kernel, np.float32)
    wok = np.asarray(wo_kernel, np.float32)

    xt = [
        [np.ascontiguousarray(x[b].T).astype(BF16) for x in (q, k, v)]
        for b in range(B)
    ]
    mt = [
        np.ascontiguousarray(mask[b].T.astype(np.float32)).astype(BF16)
        for b in range(B)
    ]
    in_maps = []
    for c in range(NCORES):
        b, g = divmod(c, GH)
        cs = slice(g * DG, (g + 1) * DG)
        wo_arr = np.ascontiguousarray(
            wok[cs, :].reshape(HPG, HD, D).transpose(1, 0, 2)
        ).astype(BF16)
        in_maps.append(
            {
                "xq_t": xt[b][0],
                "xk_t": xt[b][1],
                "xv_t": xt[b][2],
                "mask_t": mt[b],
                "wq": np.ascontiguousarray(wqk[:, cs]).astype(BF16),
                "wk": np.ascontiguousarray(wkk[:, cs]).astype(BF16),
                "wv": np.ascontiguousarray(wvk[:, cs]).astype(BF16),
                "wo": wo_arr,
                "qb": np.asarray(wq_bias, np.float32)[cs].reshape(1, DG).astype(BF16),
                "kb": np.asarray(wk_bias, np.float32)[cs].reshape(1, DG).astype(BF16),
                "vb": np.asarray(wv_bias, np.float32)[cs].reshape(1, DG).astype(BF16),
            }
        )
    return in_maps


def combine_outputs(results, wo_bias):
    outs = np.stack([np.asarray(r["out"], np.float32) for r in results])
    full = outs.reshape(B, GH, S, D).sum(axis=1)
    return (full + np.asarray(wo_bias, np.float32)[None, None, :]).astype(
        np.float32
    )


def kernel(**inputs):
    from concourse import bass_utils

    nc = _graph()
    in_maps = make_in_maps(**inputs)
    res = bass_utils.run_bass_kernel_spmd(
        nc, in_maps, core_ids=list(range(NCORES))
    )
    return combine_outputs(res.results, inputs["wo_bias"])


# revision 26
# speedup vs baseline: 1.0311x; 1.0311x over previous
# Multi-head attention (B=2, S=2048, D=1024, H=16) on 8 TRN2 NeuronCores.
#
# Sharding (hardcoded): core c in [0..8) handles batch b = c//4 and head
# group g = c%4 (4 heads = 256 output features of wq/wk/wv, 256 input rows
# of wo). Each core computes a partial output projection [S, D]; the host
# sums the 4 partials per batch and adds wo_bias (row-parallel unshard).
#
# Device-side layout choices:
#   - activations enter transposed ([D, S]) so every matmul contracts over
#     the partition axis with no on-device transposes;
#   - scores are computed transposed (S^T[k, q]) so softmax(P) feeds the
#     P@V matmul directly (contraction over k on partitions);
#   - the softmax denominator comes free as an extra ones-column appended
#     to each head's V block (output row 64 of the PV accumulation);
#   - matmuls run in float32r (full-rate fp32 path for moving dim >= 256);
#     P/V/out-proj run in bf16.
import functools
import sys

import numpy as np

try:
    import concourse  # noqa: F401
except ImportError:  # harness env without the default path
    sys.path.insert(0, "/opt/trn_rl_repo")
    sys.path.insert(0, "/opt/pypackages")

import ml_dtypes

BF16 = ml_dtypes.bfloat16

B, S, D, H = 2, 2048, 1024, 16
HD = D // H          # 64
NCORES = 8
GH = 4               # head groups (tensor-parallel)
HPG = H // GH        # heads per group = 4
DG = D // GH         # features per group = 256
P = 128              # partitions
TDIN = D // P        # 8 din tiles
SC = 4               # s-chunks of 512 for projections
CW = S // SC         # 512
QC = 2               # q-chunks of 1024 for attention
QW = S // QC         # 1024
KT = S // P          # 16 k tiles
NT2 = DG // P        # 2 dout tiles per group


def build_graph():
    """Build the SPMD Bass graph (identical on all 8 cores)."""
    from contextlib import ExitStack

    from concourse import bacc, mybir, tile

    f32 = mybir.dt.float32
    f32r = mybir.dt.float32r
    bf16 = mybir.dt.bfloat16
    EXP = mybir.ActivationFunctionType.Exp

    nc = bacc.Bacc(
        "TRN2", target_bir_lowering=False, debug=False, num_devices=NCORES
    )

    xq = nc.dram_tensor("xq_t", (D, S), bf16, kind="ExternalInput")
    xk = nc.dram_tensor("xk_t", (D, S), bf16, kind="ExternalInput")
    xv = nc.dram_tensor("xv_t", (D, S), bf16, kind="ExternalInput")
    mk = nc.dram_tensor("mask_t", (S, S), bf16, kind="ExternalInput")
    wq = nc.dram_tensor("wq", (D, DG), bf16, kind="ExternalInput")
    wk = nc.dram_tensor("wk", (D, DG), bf16, kind="ExternalInput")
    wv = nc.dram_tensor("wv", (D, DG), bf16, kind="ExternalInput")
    # wo pre-arranged host-side to [64, HPG, D] (j, h, n) so each head's
    # 64 rows sit on partitions 0..63.
    wo = nc.dram_tensor("wo", (HD, HPG, D), bf16, kind="ExternalInput")
    qb = nc.dram_tensor("qb", (1, DG), bf16, kind="ExternalInput")
    kb = nc.dram_tensor("kb", (1, DG), bf16, kind="ExternalInput")
    vb = nc.dram_tensor("vb", (1, DG), bf16, kind="ExternalInput")
    out = nc.dram_tensor("out", (S, D), bf16, kind="ExternalOutput")

    with tile.TileContext(nc) as tc, ExitStack() as ctx:
        wpool = ctx.enter_context(tc.tile_pool(name="wpool", bufs=1))
        cpool = ctx.enter_context(tc.tile_pool(name="cpool", bufs=1))
        qkpool = ctx.enter_context(tc.tile_pool(name="qk", bufs=1))
        vpool = ctx.enter_context(tc.tile_pool(name="vsb", bufs=1))
        mpool = ctx.enter_context(tc.tile_pool(name="msk", bufs=1))
        ppool = ctx.enter_context(tc.tile_pool(name="ptile", bufs=3))
        spool = ctx.enter_context(tc.tile_pool(name="small", bufs=2))
        dpool = ctx.enter_context(tc.tile_pool(name="dscr", bufs=2, space="DRAM"))
        bigps = ctx.enter_context(tc.tile_pool(name="bigps", bufs=3, space="PSUM"))
        ops_pool = ctx.enter_context(tc.tile_pool(name="ops", bufs=1, space="PSUM"))

        # ---- persistent SBUF tensors -------------------------------------
        wq_sb = wpool.tile([P, TDIN, DG], bf16)
        wk_sb = wpool.tile([P, TDIN, DG], bf16)
        wv_sb = wpool.tile([P, TDIN, DG], bf16)
        for wsb_, wdr_ in ((wq_sb, wq), (wk_sb, wk), (wv_sb, wv)):
            wr_ = wdr_.ap().rearrange("(t p) n -> p t n", p=P)
            for t_ in range(TDIN):
                nc.sync.dma_start(wsb_[:, t_, :], wr_[:, t_, :])
        wo_sb = wpool.tile([HD, HPG, D], bf16)
        nc.sync.dma_start(wo_sb[:], wo.ap())
        qb_sb = cpool.tile([1, DG], bf16)
        kb_sb = cpool.tile([1, DG], bf16)
        vb_sb = cpool.tile([1, DG], bf16)
        nc.sync.dma_start(qb_sb[:], qb.ap())
        nc.sync.dma_start(kb_sb[:], kb.ap())
        nc.sync.dma_start(vb_sb[:], vb.ap())
        # ones: row 0 used as [1, CW] rhs / [1, P] lhsT at partition 0;
        # row 64 used as [1, HD] lhsT at partition 64 (denominator bcast).
        ones2 = cpool.tile([1, CW], bf16)
        nc.vector.memset(ones2[:], 1.0)

        qT_sb = qkpool.tile([P, NT2, S], bf16)   # q projection, transposed
        kT_sb = qkpool.tile([P, NT2, S], bf16)
        # v blocks: per k-tile, per head: [v(64) | ones] -> 65 cols
        v_sb = vpool.tile([P, KT, HPG * (HD + 1)], bf16)
        nc.vector.memset(
            v_sb[:].rearrange("p s (h x) -> p s h x", h=HPG)[:, :, :, HD : HD + 1],
            1.0,
        )
        # ---- projections -------------------------------------------------
        # q, k: out qT[dout, s] = wq^T(stationary) x q^T(moving) + bias
        xpool_cm = tc.tile_pool(name="xin", bufs=3)
        xpool = xpool_cm.__enter__()
        for xdram, wsb, bias_sb, dest in (
            (xq, wq_sb, qb_sb, qT_sb),
            (xk, wk_sb, kb_sb, kT_sb),
        ):
            x_r = xdram.ap().rearrange("(t p) s -> p t s", p=P)
            for sc in range(SC):
                xch = xpool.tile([P, TDIN, CW], bf16, tag="xch")
                for half_ in range(2):
                    nc.sync.dma_start(
                        xch[:, half_ * 4 : (half_ + 1) * 4, :],
                        x_r[:, half_ * 4 : (half_ + 1) * 4, sc * CW : (sc + 1) * CW],
                    )
                for dt in range(NT2):
                    ps = bigps.tile([P, CW], f32, tag="ps")
                    for ktl in range(TDIN):
                        nc.tensor.matmul(
                            ps[:],
                            lhsT=wsb[:, ktl, dt * P : (dt + 1) * P],
                            rhs=xch[:, ktl, :],
                            start=(ktl == 0),
                            stop=False,
                        )
                    # bias: lhsT=[1,128] bias row, rhs=[1,CW] ones
                    nc.tensor.matmul(
                        ps[:],
                        lhsT=bias_sb[0:1, dt * P : (dt + 1) * P],
                        rhs=ones2[0:1, :],
                        start=False,
                        stop=True,
                    )
                    nc.scalar.copy(
                        dest[:, dt, sc * CW : (sc + 1) * CW], ps[:]
                    )
        # v: natural layout [s, dout] + bias, drained per-head with ones col
        xv_r = xv.ap().rearrange("(t p) s -> p t s", p=P)
        for sc in range(SC):
            xch = xpool.tile([P, TDIN, CW], bf16, tag="xch")
            for half_ in range(2):
                nc.sync.dma_start(
                    xch[:, half_ * 4 : (half_ + 1) * 4, :],
                    xv_r[:, half_ * 4 : (half_ + 1) * 4, sc * CW : (sc + 1) * CW],
                )
            for m in range(CW // P):
                st = sc * (CW // P) + m
                ps = bigps.tile([P, DG], f32, tag="ps")
                for ktl in range(TDIN):
                    nc.tensor.matmul(
                        ps[:],
                        lhsT=xch[:, ktl, m * P : (m + 1) * P],
                        rhs=wv_sb[:, ktl, :],
                        start=(ktl == 0),
                        stop=False,
                    )
                nc.tensor.matmul(
                    ps[:],
                    lhsT=ones2[0:1, 0:P],
                    rhs=vb_sb[:],
                    start=False,
                    stop=True,
                )
                nc.scalar.copy(
                    v_sb[:, st, :].rearrange("p (h x) -> p h x", h=HPG)[
                        :, :, 0:HD
                    ],
                    ps[:].rearrange("p (h x) -> p h x", h=HPG),
                )

        xpool_cm.__exit__(None, None, None)

        # mask load issued after projection DMAs so it doesn't hog queues
        mask_sb = mpool.tile([P, KT, S], bf16)
        mk_r = mk.ap().rearrange("(t p) q -> p t q", p=P)
        for kt in range(KT):
            nc.sync.dma_start(mask_sb[:, kt, :], mk_r[:, kt, :])

        # ---- attention ---------------------------------------------------
        # One head at a time; score psum triple-buffered so the PE can run
        # up to 3 k-tiles ahead of the exp/mask/PV chain.
        opool_sb = ctx.enter_context(tc.tile_pool(name="otn", bufs=1))
        otn_sb = opool_sb.tile([HD, HPG, S], bf16)

        def emit_outproj(st):
            osb2 = ppool.tile([P, D], bf16, tag="outsb", name=f"outsb_{st}")
            for nch in range(2):
                op_ps = bigps.tile(
                    [P, 512], f32, tag="ps", name=f"ops2_{st}_{nch}"
                )
                for h_ in range(HPG):
                    nc.tensor.matmul(
                        op_ps[:],
                        lhsT=otn_sb[:, h_, st * P : (st + 1) * P],
                        rhs=wo_sb[:, h_, nch * 512 : (nch + 1) * 512],
                        start=(h_ == 0),
                        stop=(h_ == HPG - 1),
                    )
                nc.vector.tensor_copy(
                    osb2[:, nch * 512 : (nch + 1) * 512], op_ps[:]
                )
            nc.sync.dma_start(out.ap()[st * P : (st + 1) * P, :], osb2[:])

        pending_st = []
        for qc in range(QC):
            for h in range(HPG):
                t, po = h // 2, (h % 2) * HD
                o_ps = ops_pool.tile(
                    [HD + 1, QW], f32, tag="ops", name=f"ops_{qc}_{h}"
                )
                for kt in range(KT):
                    s_ps = bigps.tile(
                        [P, QW], f32, tag="ps", name=f"sps_{qc}_{h}_{kt}"
                    )
                    for hf in range(2):
                        nc.tensor.matmul(
                            s_ps[:, hf * 512 : (hf + 1) * 512],
                            lhsT=kT_sb[po : po + HD, t, kt * P : (kt + 1) * P],
                            rhs=qT_sb[
                                po : po + HD,
                                t,
                                qc * QW + hf * 512 : qc * QW + (hf + 1) * 512,
                            ],
                            start=True,
                            stop=True,
                        )
                    pt = ppool.tile(
                        [P, QW], bf16, tag="p", name=f"pt_{qc}_{h}_{kt}"
                    )
                    nc.scalar.activation(pt[:], s_ps[:], EXP, scale=0.125)
                    meng = nc.gpsimd if kt % 4 == 3 else nc.vector
                    meng.tensor_mul(
                        pt[:], pt[:], mask_sb[:, kt, qc * QW : (qc + 1) * QW]
                    )
                    for hf in range(2):
                        nc.tensor.matmul(
                            o_ps[:, hf * 512 : (hf + 1) * 512],
                            lhsT=v_sb[:, kt, h * 65 : (h + 1) * 65],
                            rhs=pt[:, hf * 512 : (hf + 1) * 512],
                            start=(kt == 0),
                            stop=(kt == KT - 1),
                        )
                # softmax normalization (no PE): approx-recip of the
                # denominator row, DRAM-bounce broadcast, one TT multiply.
                rec65 = spool.tile([HD + 1, QW], f32, tag="rec")
                nc.vector.reciprocal_approx_fast(out=rec65[:], in_=o_ps[:])
                osb = spool.tile([HD, QW], f32, tag="osb")
                nc.vector.tensor_copy(osb[:], o_ps[0:HD, :])
                scr = dpool.tile([1, QW], f32, tag="scr", name=f"scr_{qc}_{h}")
                nc.sync.dma_start(scr[:], rec65[HD : HD + 1, :])
                rb = spool.tile([HD, QW], f32, tag="rb")
                nc.sync.dma_start(rb[:], scr[:].to_broadcast((HD, QW)))
                nc.vector.tensor_mul(
                    otn_sb[:, h, qc * QW : (qc + 1) * QW], osb[:], rb[:]
                )
                for _ in range(2):
                    if pending_st:
                        emit_outproj(pending_st.pop(0))

            pending_st.extend(range(qc * (QW // P), (qc + 1) * (QW // P)))

        for st in pending_st:
            emit_outproj(st)

    nc.compile()
    return nc


@functools.lru_cache(maxsize=1)
def _graph():
    return build_graph()


def make_in_maps(
    query, key, value, mask,
    wq_kernel, wq_bias, wk_kernel, wk_bias,
    wv_kernel, wv_bias, wo_kernel, wo_bias,
):
    q = np.asarray(query, np.float32)
    k = np.asarray(key, np.float32)
    v = np.asarray(value, np.float32)
    mask = np.asarray(mask)
    wqk = np.asarray(wq_kernel, np.float32)
    wkk = np.asarray(wk_kernel, np.float32)
    wvk = np.asarray(wv_kernel, np.float32)
    wok = np.asarray(wo_kernel, np.float32)

    xt = [
        [np.ascontiguousarray(x[b].T).astype(BF16) for x in (q, k, v)]
        for b in range(B)
    ]
    mt = [
        np.ascontiguousarray(mask[b].T.astype(np.float32)).astype(BF16)
        for b in range(B)
    ]
    in_maps = []
    for c in range(NCORES):
        b, g = divmod(c, GH)
        cs = slice(g * DG, (g + 1) * DG)
        wo_arr = np.ascontiguousarray(
            wok[cs, :].reshape(HPG, HD, D).transpose(1, 0, 2)
        ).astype(BF16)
        in_maps.append(
            {
                "xq_t": xt[b][0],
                "xk_t": xt[b][1],
                "xv_t": xt[b][2],
                "mask_t": mt[b],
                "wq": np.ascontiguousarray(wqk[:, cs]).astype(BF16),
                "wk": np.ascontiguousarray(wkk[:, cs]).astype(BF16),
                "wv": np.ascontiguousarray(wvk[:, cs]).astype(BF16),
                "wo": wo_arr,
                "qb": np.asarray(wq_bias, np.float32)[cs].reshape(1, DG).astype(BF16),
                "kb": np.asarray(wk_bias, np.float32)[cs].reshape(1, DG).astype(BF16),
                "vb": np.asarray(wv_bias, np.float32)[cs].reshape(1, DG).astype(BF16),
            }
        )
    return in_maps


def combine_outputs(results, wo_bias):
    outs = np.stack([np.asarray(r["out"], np.float32) for r in results])
    full = outs.reshape(B, GH, S, D).sum(axis=1)
    return (full + np.asarray(wo_bias, np.float32)[None, None, :]).astype(
        np.float32
    )


def kernel(**inputs):
    from concourse import bass_utils

    nc = _graph()
    in_maps = make_in_maps(**inputs)
    res = bass_utils.run_bass_kernel_spmd(
        nc, in_maps, core_ids=list(range(NCORES))
    )
    return combine_outputs(res.results, inputs["wo_bias"])


# revision 27
# speedup vs baseline: 1.0360x; 1.0047x over previous
# Multi-head attention (B=2, S=2048, D=1024, H=16) on 8 TRN2 NeuronCores.
#
# Sharding (hardcoded): core c in [0..8) handles batch b = c//4 and head
# group g = c%4 (4 heads = 256 output features of wq/wk/wv, 256 input rows
# of wo). Each core computes a partial output projection [S, D]; the host
# sums the 4 partials per batch and adds wo_bias (row-parallel unshard).
#
# Device-side layout choices:
#   - activations enter transposed ([D, S]) so every matmul contracts over
#     the partition axis with no on-device transposes;
#   - scores are computed transposed (S^T[k, q]) so softmax(P) feeds the
#     P@V matmul directly (contraction over k on partitions);
#   - the softmax denominator comes free as an extra ones-column appended
#     to each head's V block (output row 64 of the PV accumulation);
#   - matmuls run in float32r (full-rate fp32 path for moving dim >= 256);
#     P/V/out-proj run in bf16.
import functools
import sys

import numpy as np

try:
    import concourse  # noqa: F401
except ImportError:  # harness env without the default path
    sys.path.insert(0, "/opt/trn_rl_repo")
    sys.path.insert(0, "/opt/pypackages")

import ml_dtypes

BF16 = ml_dtypes.bfloat16

B, S, D, H = 2, 2048, 1024, 16
HD = D // H          # 64
NCORES = 8
GH = 4               # head groups (tensor-parallel)
HPG = H // GH        # heads per group = 4
DG = D // GH         # features per group = 256
P = 128              # partitions
TDIN = D // P        # 8 din tiles
SC = 4               # s-chunks of 512 for projections
CW = S // SC         # 512
QC = 2               # q-chunks of 1024 for attention
QW = S // QC         # 1024
KT = S // P          # 16 k tiles
NT2 = DG // P        # 2 dout tiles per group


def build_graph():
    """Build the SPMD Bass graph (identical on all 8 cores)."""
    from contextlib import ExitStack

    from concourse import bacc, mybir, tile

    f32 = mybir.dt.float32
    f32r = mybir.dt.float32r
    bf16 = mybir.dt.bfloat16
    EXP = mybir.ActivationFunctionType.Exp

    nc = bacc.Bacc(
        "TRN2", target_bir_lowering=False, debug=False, num_devices=NCORES
    )

    xq = nc.dram_tensor("xq_t", (P, TDIN, S), bf16, kind="ExternalInput")
    xk = nc.dram_tensor("xk_t", (P, TDIN, S), bf16, kind="ExternalInput")
    xv = nc.dram_tensor("xv_t", (P, TDIN, S), bf16, kind="ExternalInput")
    mk = nc.dram_tensor("mask_t", (S, S), bf16, kind="ExternalInput")
    wq = nc.dram_tensor("wq", (P, TDIN, DG), bf16, kind="ExternalInput")
    wk = nc.dram_tensor("wk", (P, TDIN, DG), bf16, kind="ExternalInput")
    wv = nc.dram_tensor("wv", (P, TDIN, DG), bf16, kind="ExternalInput")
    # wo pre-arranged host-side to [64, HPG, D] (j, h, n) so each head's
    # 64 rows sit on partitions 0..63.
    wo = nc.dram_tensor("wo", (HD, HPG, D), bf16, kind="ExternalInput")
    qb = nc.dram_tensor("qb", (1, DG), bf16, kind="ExternalInput")
    kb = nc.dram_tensor("kb", (1, DG), bf16, kind="ExternalInput")
    vb = nc.dram_tensor("vb", (1, DG), bf16, kind="ExternalInput")
    out = nc.dram_tensor("out", (S, D), bf16, kind="ExternalOutput")

    with tile.TileContext(nc) as tc, ExitStack() as ctx:
        wpool = ctx.enter_context(tc.tile_pool(name="wpool", bufs=1))
        cpool = ctx.enter_context(tc.tile_pool(name="cpool", bufs=1))
        qkpool = ctx.enter_context(tc.tile_pool(name="qk", bufs=1))
        vpool = ctx.enter_context(tc.tile_pool(name="vsb", bufs=1))
        mpool = ctx.enter_context(tc.tile_pool(name="msk", bufs=1))
        ppool = ctx.enter_context(tc.tile_pool(name="ptile", bufs=3))
        spool = ctx.enter_context(tc.tile_pool(name="small", bufs=2))
        dpool = ctx.enter_context(tc.tile_pool(name="dscr", bufs=2, space="DRAM"))
        bigps = ctx.enter_context(tc.tile_pool(name="bigps", bufs=3, space="PSUM"))
        ops_pool = ctx.enter_context(tc.tile_pool(name="ops", bufs=1, space="PSUM"))

        # ---- persistent SBUF tensors -------------------------------------
        wq_sb = wpool.tile([P, TDIN, DG], bf16)
        wk_sb = wpool.tile([P, TDIN, DG], bf16)
        wv_sb = wpool.tile([P, TDIN, DG], bf16)
        for wsb_, wdr_ in ((wq_sb, wq), (wk_sb, wk), (wv_sb, wv)):
            nc.sync.dma_start(wsb_[:], wdr_.ap())
        wo_sb = wpool.tile([HD, HPG, D], bf16)
        nc.sync.dma_start(wo_sb[:], wo.ap())
        qb_sb = cpool.tile([1, DG], bf16)
        kb_sb = cpool.tile([1, DG], bf16)
        vb_sb = cpool.tile([1, DG], bf16)
        nc.sync.dma_start(qb_sb[:], qb.ap())
        nc.sync.dma_start(kb_sb[:], kb.ap())
        nc.sync.dma_start(vb_sb[:], vb.ap())
        # ones: row 0 used as [1, CW] rhs / [1, P] lhsT at partition 0;
        # row 64 used as [1, HD] lhsT at partition 64 (denominator bcast).
        ones2 = cpool.tile([1, CW], bf16)
        nc.vector.memset(ones2[:], 1.0)

        qT_sb = qkpool.tile([P, NT2, S], bf16)   # q projection, transposed
        kT_sb = qkpool.tile([P, NT2, S], bf16)
        # v blocks: per k-tile, per head: [v(64) | ones] -> 65 cols
        v_sb = vpool.tile([P, KT, HPG * (HD + 1)], bf16)
        nc.vector.memset(
            v_sb[:].rearrange("p s (h x) -> p s h x", h=HPG)[:, :, :, HD : HD + 1],
            1.0,
        )
        # ---- projections -------------------------------------------------
        # q, k: out qT[dout, s] = wq^T(stationary) x q^T(moving) + bias
        xpool_cm = tc.tile_pool(name="xin", bufs=2)
        xpool = xpool_cm.__enter__()
        NCH = S // 1024
        for xdram, wsb, bias_sb, dest in (
            (xq, wq_sb, qb_sb, qT_sb),
            (xk, wk_sb, kb_sb, kT_sb),
        ):
            for sc in range(NCH):
                xch = xpool.tile([P, TDIN, 1024], bf16, tag="xch")
                nc.sync.dma_start(
                    xch[:], xdram.ap()[:, :, sc * 1024 : (sc + 1) * 1024]
                )
                for half in range(2):
                    s0 = sc * 1024 + half * 512
                    for dt in range(NT2):
                        ps = bigps.tile(
                            [P, CW], f32, tag="ps", name=f"pj_{sc}_{half}_{dt}"
                        )
                        for ktl in range(TDIN):
                            nc.tensor.matmul(
                                ps[:],
                                lhsT=wsb[:, ktl, dt * P : (dt + 1) * P],
                                rhs=xch[:, ktl, half * 512 : (half + 1) * 512],
                                start=(ktl == 0),
                                stop=False,
                            )
                        nc.tensor.matmul(
                            ps[:],
                            lhsT=bias_sb[0:1, dt * P : (dt + 1) * P],
                            rhs=ones2[0:1, :],
                            start=False,
                            stop=True,
                        )
                        nc.scalar.copy(dest[:, dt, s0 : s0 + 512], ps[:])
        # v: natural layout [s, dout] + bias, drained per-head with ones col
        for sc in range(NCH):
            xch = xpool.tile([P, TDIN, 1024], bf16, tag="xch")
            nc.sync.dma_start(
                xch[:], xv.ap()[:, :, sc * 1024 : (sc + 1) * 1024]
            )
            for m in range(1024 // P):
                st = sc * (1024 // P) + m
                ps = bigps.tile([P, DG], f32, tag="ps", name=f"pv_{sc}_{m}")
                for ktl in range(TDIN):
                    nc.tensor.matmul(
                        ps[:],
                        lhsT=xch[:, ktl, m * P : (m + 1) * P],
                        rhs=wv_sb[:, ktl, :],
                        start=(ktl == 0),
                        stop=False,
                    )
                nc.tensor.matmul(
                    ps[:],
                    lhsT=ones2[0:1, 0:P],
                    rhs=vb_sb[:],
                    start=False,
                    stop=True,
                )
                nc.scalar.copy(
                    v_sb[:, st, :].rearrange("p (h x) -> p h x", h=HPG)[
                        :, :, 0:HD
                    ],
                    ps[:].rearrange("p (h x) -> p h x", h=HPG),
                )
        xpool_cm.__exit__(None, None, None)

        # mask load issued after projection DMAs so it doesn't hog queues
        mask_sb = mpool.tile([P, KT, S], bf16)
        mk_r = mk.ap().rearrange("(t p) q -> p t q", p=P)
        for kt in range(KT):
            nc.sync.dma_start(mask_sb[:, kt, :], mk_r[:, kt, :])

        # ---- attention ---------------------------------------------------
        # One head at a time; score psum triple-buffered so the PE can run
        # up to 3 k-tiles ahead of the exp/mask/PV chain.
        opool_sb = ctx.enter_context(tc.tile_pool(name="otn", bufs=1))
        otn_sb = opool_sb.tile([HD, HPG, S], bf16)

        def emit_outproj(st):
            osb2 = ppool.tile([P, D], bf16, tag="outsb", name=f"outsb_{st}")
            for nch in range(2):
                op_ps = bigps.tile(
                    [P, 512], f32, tag="ps", name=f"ops2_{st}_{nch}"
                )
                for h_ in range(HPG):
                    nc.tensor.matmul(
                        op_ps[:],
                        lhsT=otn_sb[:, h_, st * P : (st + 1) * P],
                        rhs=wo_sb[:, h_, nch * 512 : (nch + 1) * 512],
                        start=(h_ == 0),
                        stop=(h_ == HPG - 1),
                    )
                nc.vector.tensor_copy(
                    osb2[:, nch * 512 : (nch + 1) * 512], op_ps[:]
                )
            nc.sync.dma_start(out.ap()[st * P : (st + 1) * P, :], osb2[:])

        pending_st = []
        for qc in range(QC):
            for h in range(HPG):
                t, po = h // 2, (h % 2) * HD
                o_ps = ops_pool.tile(
                    [HD + 1, QW], f32, tag="ops", name=f"ops_{qc}_{h}"
                )
                for kt in range(KT):
                    s_ps = bigps.tile(
                        [P, QW], f32, tag="ps", name=f"sps_{qc}_{h}_{kt}"
                    )
                    for hf in range(2):
                        nc.tensor.matmul(
                            s_ps[:, hf * 512 : (hf + 1) * 512],
                            lhsT=kT_sb[po : po + HD, t, kt * P : (kt + 1) * P],
                            rhs=qT_sb[
                                po : po + HD,
                                t,
                                qc * QW + hf * 512 : qc * QW + (hf + 1) * 512,
                            ],
                            start=True,
                            stop=True,
                        )
                    pt = ppool.tile(
                        [P, QW], bf16, tag="p", name=f"pt_{qc}_{h}_{kt}"
                    )
                    nc.scalar.activation(pt[:], s_ps[:], EXP, scale=0.125)
                    meng = nc.gpsimd if kt % 4 == 3 else nc.vector
                    meng.tensor_mul(
                        pt[:], pt[:], mask_sb[:, kt, qc * QW : (qc + 1) * QW]
                    )
                    for hf in range(2):
                        nc.tensor.matmul(
                            o_ps[:, hf * 512 : (hf + 1) * 512],
                            lhsT=v_sb[:, kt, h * 65 : (h + 1) * 65],
                            rhs=pt[:, hf * 512 : (hf + 1) * 512],
                            start=(kt == 0),
                            stop=(kt == KT - 1),
                        )
                # softmax normalization (no PE): approx-recip of the
                # denominator row, DRAM-bounce broadcast, one TT multiply.
                rec65 = spool.tile([HD + 1, QW], f32, tag="rec")
                nc.vector.reciprocal_approx_fast(out=rec65[:], in_=o_ps[:])
                osb = spool.tile([HD, QW], f32, tag="osb")
                nc.vector.tensor_copy(osb[:], o_ps[0:HD, :])
                scr = dpool.tile([1, QW], f32, tag="scr", name=f"scr_{qc}_{h}")
                nc.sync.dma_start(scr[:], rec65[HD : HD + 1, :])
                rb = spool.tile([HD, QW], f32, tag="rb")
                nc.sync.dma_start(rb[:], scr[:].to_broadcast((HD, QW)))
                nc.vector.tensor_mul(
                    otn_sb[:, h, qc * QW : (qc + 1) * QW], osb[:], rb[:]
                )
                for _ in range(2):
                    if pending_st:
                        emit_outproj(pending_st.pop(0))

            pending_st.extend(range(qc * (QW // P), (qc + 1) * (QW // P)))

        for st in pending_st:
            emit_outproj(st)

    nc.compile()
    return nc


@functools.lru_cache(maxsize=1)
def _graph():
    return build_graph()


def make_in_maps(
    query, key, value, mask,
    wq_kernel, wq_bias, wk_kernel, wk_bias,
    wv_kernel, wv_bias, wo_kernel, wo_bias,
):
    q = np.asarray(query, np.float32)
    k = np.asarray(key, np.float32)
    v = np.asarray(value, np.float32)
    mask = np.asarray(mask)
    wqk = np.asarray(wq_kernel, np.float32)
    wkk = np.asarray(wk_kernel, np.float32)
    wvk = np.asarray(wv_kernel, np.float32)
    wok = np.asarray(wo_kernel, np.float32)

    def tile_x(a):  # [S, D] -> [P, TDIN, S] pre-tiled transpose
        return np.ascontiguousarray(
            a.T.reshape(TDIN, P, S).transpose(1, 0, 2)
        ).astype(BF16)

    xt = [[tile_x(x[b]) for x in (q, k, v)] for b in range(B)]
    mt = [
        np.ascontiguousarray(mask[b].T.astype(np.float32)).astype(BF16)
        for b in range(B)
    ]
    in_maps = []
    for c in range(NCORES):
        b, g = divmod(c, GH)
        cs = slice(g * DG, (g + 1) * DG)
        wo_arr = np.ascontiguousarray(
            wok[cs, :].reshape(HPG, HD, D).transpose(1, 0, 2)
        ).astype(BF16)
        in_maps.append(
            {
                "xq_t": xt[b][0],
                "xk_t": xt[b][1],
                "xv_t": xt[b][2],
                "mask_t": mt[b],
                "wq": np.ascontiguousarray(wqk[:, cs].reshape(TDIN, P, DG).transpose(1, 0, 2)).astype(BF16),
                "wk": np.ascontiguousarray(wkk[:, cs].reshape(TDIN, P, DG).transpose(1, 0, 2)).astype(BF16),
                "wv": np.ascontiguousarray(wvk[:, cs].reshape(TDIN, P, DG).transpose(1, 0, 2)).astype(BF16),
                "wo": wo_arr,
                "qb": np.asarray(wq_bias, np.float32)[cs].reshape(1, DG).astype(BF16),
                "kb": np.asarray(wk_bias, np.float32)[cs].reshape(1, DG).astype(BF16),
                "vb": np.asarray(wv_bias, np.float32)[cs].reshape(1, DG).astype(BF16),
            }
        )
    return in_maps


def combine_outputs(results, wo_bias):
    outs = np.stack([np.asarray(r["out"], np.float32) for r in results])
    full = outs.reshape(B, GH, S, D).sum(axis=1)
    return (full + np.asarray(wo_bias, np.float32)[None, None, :]).astype(
        np.float32
    )


def kernel(**inputs):
    from concourse import bass_utils

    nc = _graph()
    in_maps = make_in_maps(**inputs)
    res = bass_utils.run_bass_kernel_spmd(
        nc, in_maps, core_ids=list(range(NCORES))
    )
    return combine_outputs(res.results, inputs["wo_bias"])


# revision 29
# speedup vs baseline: 1.0694x; 1.0323x over previous
# Multi-head attention (B=2, S=2048, D=1024, H=16) on 8 TRN2 NeuronCores.
#
# Sharding (hardcoded): core c in [0..8) handles batch b = c//4 and head
# group g = c%4 (4 heads = 256 output features of wq/wk/wv, 256 input rows
# of wo). Each core computes a partial output projection [S, D]; the host
# sums the 4 partials per batch and adds wo_bias (row-parallel unshard).
#
# Device-side layout choices:
#   - activations enter transposed ([D, S]) so every matmul contracts over
#     the partition axis with no on-device transposes;
#   - scores are computed transposed (S^T[k, q]) so softmax(P) feeds the
#     P@V matmul directly (contraction over k on partitions);
#   - the softmax denominator comes free as an extra ones-column appended
#     to each head's V block (output row 64 of the PV accumulation);
#   - matmuls run in float32r (full-rate fp32 path for moving dim >= 256);
#     P/V/out-proj run in bf16.
import functools
import sys

import numpy as np

try:
    import concourse  # noqa: F401
except ImportError:  # harness env without the default path
    sys.path.insert(0, "/opt/trn_rl_repo")
    sys.path.insert(0, "/opt/pypackages")

import ml_dtypes

BF16 = ml_dtypes.bfloat16

B, S, D, H = 2, 2048, 1024, 16
HD = D // H          # 64
NCORES = 8
GH = 4               # head groups (tensor-parallel)
HPG = H // GH        # heads per group = 4
DG = D // GH         # features per group = 256
P = 128              # partitions
TDIN = D // P        # 8 din tiles
SC = 4               # s-chunks of 512 for projections
CW = S // SC         # 512
QC = 2               # q-chunks of 1024 for attention
QW = S // QC         # 1024
KT = S // P          # 16 k tiles
NT2 = DG // P        # 2 dout tiles per group


def build_graph():
    """Build the SPMD Bass graph (identical on all 8 cores)."""
    from contextlib import ExitStack

    from concourse import bacc, mybir, tile

    f32 = mybir.dt.float32
    f32r = mybir.dt.float32r
    bf16 = mybir.dt.bfloat16
    EXP = mybir.ActivationFunctionType.Exp

    nc = bacc.Bacc(
        "TRN2", target_bir_lowering=False, debug=False, num_devices=NCORES
    )

    xq = nc.dram_tensor("xq_t", (P, TDIN, S), bf16, kind="ExternalInput")
    xk = nc.dram_tensor("xk_t", (P, TDIN, S), bf16, kind="ExternalInput")
    xv = nc.dram_tensor("xv_t", (P, TDIN, S), bf16, kind="ExternalInput")
    mk = nc.dram_tensor("mask_t", (S, S), bf16, kind="ExternalInput")
    wq = nc.dram_tensor("wq", (P, TDIN, DG), bf16, kind="ExternalInput")
    wk = nc.dram_tensor("wk", (P, TDIN, DG), bf16, kind="ExternalInput")
    wv = nc.dram_tensor("wv", (P, TDIN, DG), bf16, kind="ExternalInput")
    # wo pre-arranged host-side to [64, HPG, D] (j, h, n) so each head's
    # 64 rows sit on partitions 0..63.
    wo = nc.dram_tensor("wo", (HD, HPG, D), bf16, kind="ExternalInput")
    qb = nc.dram_tensor("qb", (1, DG), bf16, kind="ExternalInput")
    kb = nc.dram_tensor("kb", (1, DG), bf16, kind="ExternalInput")
    vb = nc.dram_tensor("vb", (1, DG), bf16, kind="ExternalInput")
    out = nc.dram_tensor("out", (S, D), bf16, kind="ExternalOutput")

    with tile.TileContext(nc) as tc, ExitStack() as ctx:
        wpool = ctx.enter_context(tc.tile_pool(name="wpool", bufs=1))
        cpool = ctx.enter_context(tc.tile_pool(name="cpool", bufs=1))
        qkpool = ctx.enter_context(tc.tile_pool(name="qk", bufs=1))
        vpool = ctx.enter_context(tc.tile_pool(name="vsb", bufs=1))
        mpool = ctx.enter_context(tc.tile_pool(name="msk", bufs=1))
        ppool = ctx.enter_context(tc.tile_pool(name="ptile", bufs=3))
        spool = ctx.enter_context(tc.tile_pool(name="small", bufs=2))
        dpool = ctx.enter_context(tc.tile_pool(name="dscr", bufs=2, space="DRAM"))
        bigps = ctx.enter_context(tc.tile_pool(name="bigps", bufs=3, space="PSUM"))
        ops_pool = ctx.enter_context(tc.tile_pool(name="ops", bufs=1, space="PSUM"))

        # ---- persistent SBUF tensors -------------------------------------
        wq_sb = wpool.tile([P, TDIN, DG], bf16)
        wk_sb = wpool.tile([P, TDIN, DG], bf16)
        wv_sb = wpool.tile([P, TDIN, DG], bf16)
        for wsb_, wdr_ in ((wq_sb, wq), (wk_sb, wk), (wv_sb, wv)):
            nc.sync.dma_start(wsb_[:], wdr_.ap())
        wo_sb = wpool.tile([HD, HPG, D], bf16)
        nc.sync.dma_start(wo_sb[:], wo.ap())
        qb_sb = cpool.tile([1, DG], bf16)
        kb_sb = cpool.tile([1, DG], bf16)
        vb_sb = cpool.tile([1, DG], bf16)
        nc.sync.dma_start(qb_sb[:], qb.ap())
        nc.sync.dma_start(kb_sb[:], kb.ap())
        nc.sync.dma_start(vb_sb[:], vb.ap())
        # ones: row 0 used as [1, CW] rhs / [1, P] lhsT at partition 0;
        # row 64 used as [1, HD] lhsT at partition 64 (denominator bcast).
        ones2 = cpool.tile([1, CW], bf16)
        nc.vector.memset(ones2[:], 1.0)

        qT_sb = qkpool.tile([P, NT2, S], bf16)   # q projection, transposed
        kT_sb = qkpool.tile([P, NT2, S], bf16)
        # v blocks: per k-tile, per head: [v(64) | ones] -> 65 cols
        v_sb = vpool.tile([P, KT, HPG * (HD + 1)], bf16)
        nc.vector.memset(
            v_sb[:].rearrange("p s (h x) -> p s h x", h=HPG)[:, :, :, HD : HD + 1],
            1.0,
        )
        # ---- projections -------------------------------------------------
        # q, k: out qT[dout, s] = wq^T(stationary) x q^T(moving) + bias
        xpool_cm = tc.tile_pool(name="xin", bufs=2)
        xpool = xpool_cm.__enter__()
        NCH = S // 1024
        for xdram, wsb, bias_sb, dest in (
            (xq, wq_sb, qb_sb, qT_sb),
            (xk, wk_sb, kb_sb, kT_sb),
        ):
            for sc in range(NCH):
                xch = xpool.tile([P, TDIN, 1024], bf16, tag="xch")
                nc.sync.dma_start(
                    xch[:], xdram.ap()[:, :, sc * 1024 : (sc + 1) * 1024]
                )
                for half in range(2):
                    s0 = sc * 1024 + half * 512
                    for dt in range(NT2):
                        ps = bigps.tile(
                            [P, CW], f32, tag="ps", name=f"pj_{sc}_{half}_{dt}"
                        )
                        for ktl in range(TDIN):
                            nc.tensor.matmul(
                                ps[:],
                                lhsT=wsb[:, ktl, dt * P : (dt + 1) * P],
                                rhs=xch[:, ktl, half * 512 : (half + 1) * 512],
                                start=(ktl == 0),
                                stop=False,
                            )
                        nc.tensor.matmul(
                            ps[:],
                            lhsT=bias_sb[0:1, dt * P : (dt + 1) * P],
                            rhs=ones2[0:1, :],
                            start=False,
                            stop=True,
                        )
                        nc.scalar.copy(dest[:, dt, s0 : s0 + 512], ps[:])
        # v: natural layout [s, dout] + bias, drained per-head with ones col
        for sc in range(NCH):
            xch = xpool.tile([P, TDIN, 1024], bf16, tag="xch")
            nc.sync.dma_start(
                xch[:], xv.ap()[:, :, sc * 1024 : (sc + 1) * 1024]
            )
            for m in range(1024 // P):
                st = sc * (1024 // P) + m
                ps = bigps.tile([P, DG], f32, tag="ps", name=f"pv_{sc}_{m}")
                for ktl in range(TDIN):
                    nc.tensor.matmul(
                        ps[:],
                        lhsT=xch[:, ktl, m * P : (m + 1) * P],
                        rhs=wv_sb[:, ktl, :],
                        start=(ktl == 0),
                        stop=False,
                    )
                nc.tensor.matmul(
                    ps[:],
                    lhsT=ones2[0:1, 0:P],
                    rhs=vb_sb[:],
                    start=False,
                    stop=True,
                )
                nc.scalar.copy(
                    v_sb[:, st, :].rearrange("p (h x) -> p h x", h=HPG)[
                        :, :, 0:HD
                    ],
                    ps[:].rearrange("p (h x) -> p h x", h=HPG),
                )
        xpool_cm.__exit__(None, None, None)

        # mask load issued after projection DMAs so it doesn't hog queues
        mask_sb = mpool.tile([P, KT, S], bf16)
        mk_r = mk.ap().rearrange("(t p) q -> p t q", p=P)
        for kt in range(KT):
            nc.sync.dma_start(mask_sb[:, kt, :], mk_r[:, kt, :])

        # ---- attention ---------------------------------------------------
        # One head at a time; score psum triple-buffered so the PE can run
        # up to 3 k-tiles ahead of the exp/mask/PV chain.
        opool_sb = ctx.enter_context(tc.tile_pool(name="otn", bufs=1))
        otn_sb = opool_sb.tile([HD, HPG, S], bf16)

        def emit_outproj(st):
            osb2 = ppool.tile([P, D], bf16, tag="outsb", name=f"outsb_{st}")
            for nch in range(2):
                op_ps = bigps.tile(
                    [P, 512], f32, tag="ps", name=f"ops2_{st}_{nch}"
                )
                for h_ in range(HPG):
                    nc.tensor.matmul(
                        op_ps[:],
                        lhsT=otn_sb[:, h_, st * P : (st + 1) * P],
                        rhs=wo_sb[:, h_, nch * 512 : (nch + 1) * 512],
                        start=(h_ == 0),
                        stop=(h_ == HPG - 1),
                    )
                nc.vector.tensor_copy(
                    osb2[:, nch * 512 : (nch + 1) * 512], op_ps[:]
                )
            nc.sync.dma_start(out.ap()[st * P : (st + 1) * P, :], osb2[:])

        pending_st = []
        for qc in range(QC):
            for h in range(HPG):
                t, po = h // 2, (h % 2) * HD
                o_ps = ops_pool.tile(
                    [HD + 1, QW], f32, tag="ops", name=f"ops_{qc}_{h}"
                )
                for kt in range(KT):
                    s_ps = bigps.tile(
                        [P, QW], f32, tag="ps", name=f"sps_{qc}_{h}_{kt}"
                    )
                    for hf in range(2):
                        nc.tensor.matmul(
                            s_ps[:, hf * 512 : (hf + 1) * 512],
                            lhsT=kT_sb[po : po + HD, t, kt * P : (kt + 1) * P],
                            rhs=qT_sb[
                                po : po + HD,
                                t,
                                qc * QW + hf * 512 : qc * QW + (hf + 1) * 512,
                            ],
                            start=True,
                            stop=True,
                        )
                    pt = ppool.tile(
                        [P, QW], bf16, tag="p", name=f"pt_{qc}_{h}_{kt}"
                    )
                    nc.scalar.activation(pt[:], s_ps[:], EXP, scale=0.125)
                    meng = nc.gpsimd if kt % 8 == 7 else nc.vector
                    meng.tensor_mul(
                        pt[:], pt[:], mask_sb[:, kt, qc * QW : (qc + 1) * QW]
                    )
                    for hf in range(2):
                        nc.tensor.matmul(
                            o_ps[:, hf * 512 : (hf + 1) * 512],
                            lhsT=v_sb[:, kt, h * 65 : (h + 1) * 65],
                            rhs=pt[:, hf * 512 : (hf + 1) * 512],
                            start=(kt == 0),
                            stop=(kt == KT - 1),
                        )
                # softmax normalization (no PE): approx-recip of the
                # denominator row, DRAM-bounce broadcast, one TT multiply.
                rec65 = spool.tile([HD + 1, QW], f32, tag="rec")
                nc.vector.reciprocal_approx_fast(out=rec65[:], in_=o_ps[:])
                osb = spool.tile([HD, QW], f32, tag="osb")
                nc.vector.tensor_copy(osb[:], o_ps[0:HD, :])
                scr = dpool.tile([1, QW], f32, tag="scr", name=f"scr_{qc}_{h}")
                nc.sync.dma_start(scr[:], rec65[HD : HD + 1, :])
                rb = spool.tile([HD, QW], f32, tag="rb")
                nc.sync.dma_start(rb[:], scr[:].to_broadcast((HD, QW)))
                nc.vector.tensor_mul(
                    otn_sb[:, h, qc * QW : (qc + 1) * QW], osb[:], rb[:]
                )
                for _ in range(2):
                    if pending_st:
                        emit_outproj(pending_st.pop(0))

            pending_st.extend(range(qc * (QW // P), (qc + 1) * (QW // P)))

        for st in pending_st:
            emit_outproj(st)

    nc.compile()
    return nc


@functools.lru_cache(maxsize=1)
def _graph():
    return build_graph()


def make_in_maps(
    query, key, value, mask,
    wq_kernel, wq_bias, wk_kernel, wk_bias,
    wv_kernel, wv_bias, wo_kernel, wo_bias,
):
    q = np.asarray(query, np.float32)
    k = np.asarray(key, np.float32)
    v = np.asarray(value, np.float32)
    mask = np.asarray(mask)
    wqk = np.asarray(wq_kernel, np.float32)
    wkk = np.asarray(wk_kernel, np.float32)
    wvk = np.asarray(wv_kernel, np.float32)
    wok = np.asarray(wo_kernel, np.float32)

    def tile_x(a):  # [S, D] -> [P, TDIN, S] pre-tiled transpose
        return np.ascontiguousarray(
            a.T.reshape(TDIN, P, S).transpose(1, 0, 2)
        ).astype(BF16)

    xt = [[tile_x(x[b]) for x in (q, k, v)] for b in range(B)]
    mt = [
        np.ascontiguousarray(mask[b].T.astype(np.float32)).astype(BF16)
        for b in range(B)
    ]
    in_maps = []
    for c in range(NCORES):
        b, g = divmod(c, GH)
        cs = slice(g * DG, (g + 1) * DG)
        wo_arr = np.ascontiguousarray(
            wok[cs, :].reshape(HPG, HD, D).transpose(1, 0, 2)
        ).astype(BF16)
        in_maps.append(
            {
                "xq_t": xt[b][0],
                "xk_t": xt[b][1],
                "xv_t": xt[b][2],
                "mask_t": mt[b],
                "wq": np.ascontiguousarray(wqk[:, cs].reshape(TDIN, P, DG).transpose(1, 0, 2)).astype(BF16),
                "wk": np.ascontiguousarray(wkk[:, cs].reshape(TDIN, P, DG).transpose(1, 0, 2)).astype(BF16),
                "wv": np.ascontiguousarray(wvk[:, cs].reshape(TDIN, P, DG).transpose(1, 0, 2)).astype(BF16),
                "wo": wo_arr,
                "qb": np.asarray(wq_bias, np.float32)[cs].reshape(1, DG).astype(BF16),
                "kb": np.asarray(wk_bias, np.float32)[cs].reshape(1, DG).astype(BF16),
                "vb": np.asarray(wv_bias, np.float32)[cs].reshape(1, DG).astype(BF16),
            }
        )
    return in_maps


def combine_outputs(results, wo_bias):
    outs = np.stack([np.asarray(r["out"], np.float32) for r in results])
    full = outs.reshape(B, GH, S, D).sum(axis=1)
    return (full + np.asarray(wo_bias, np.float32)[None, None, :]).astype(
        np.float32
    )


def kernel(**inputs):
    from concourse import bass_utils

    nc = _graph()
    in_maps = make_in_maps(**inputs)
    res = bass_utils.run_bass_kernel_spmd(
        nc, in_maps, core_ids=list(range(NCORES))
    )
    return combine_outputs(res.results, inputs["wo_bias"])


# revision 30
# speedup vs baseline: 1.2061x; 1.1278x over previous
# Multi-head attention (B=2, S=2048, D=1024, H=16) on 8 TRN2 NeuronCores.
#
# Sharding (hardcoded): core c in [0..8) handles batch b = c//4 and head
# group g = c%4 (4 heads = 256 output features of wq/wk/wv, 256 input rows
# of wo). Each core computes a partial output projection [S, D]; the host
# sums the 4 partials per batch and adds wo_bias (row-parallel unshard).
#
# Device-side layout choices:
#   - activations enter transposed ([D, S]) so every matmul contracts over
#     the partition axis with no on-device transposes;
#   - scores are computed transposed (S^T[k, q]) so softmax(P) feeds the
#     P@V matmul directly (contraction over k on partitions);
#   - the softmax denominator comes free as an extra ones-column appended
#     to each head's V block (output row 64 of the PV accumulation);
#   - matmuls run in float32r (full-rate fp32 path for moving dim >= 256);
#     P/V/out-proj run in bf16.
import functools
import sys

import numpy as np

try:
    import concourse  # noqa: F401
except ImportError:  # harness env without the default path
    sys.path.insert(0, "/opt/trn_rl_repo")
    sys.path.insert(0, "/opt/pypackages")

import ml_dtypes

BF16 = ml_dtypes.bfloat16

B, S, D, H = 2, 2048, 1024, 16
HD = D // H          # 64
NCORES = 8
GH = 4               # head groups (tensor-parallel)
HPG = H // GH        # heads per group = 4
DG = D // GH         # features per group = 256
P = 128              # partitions
TDIN = D // P        # 8 din tiles
SC = 4               # s-chunks of 512 for projections
CW = S // SC         # 512
QC = 2               # q-chunks of 1024 for attention
QW = S // QC         # 1024
KT = S // P          # 16 k tiles
NT2 = DG // P        # 2 dout tiles per group


def build_graph():
    """Build the SPMD Bass graph (identical on all 8 cores)."""
    from contextlib import ExitStack

    from concourse import bacc, mybir, tile

    f32 = mybir.dt.float32
    f32r = mybir.dt.float32r
    bf16 = mybir.dt.bfloat16
    EXP = mybir.ActivationFunctionType.Exp

    nc = bacc.Bacc(
        "TRN2", target_bir_lowering=False, debug=False, num_devices=NCORES
    )

    xq = nc.dram_tensor("xq_t", (P, TDIN, S), bf16, kind="ExternalInput")
    xk = nc.dram_tensor("xk_t", (P, TDIN, S), bf16, kind="ExternalInput")
    xv = nc.dram_tensor("xv_t", (P, TDIN, S), bf16, kind="ExternalInput")
    mk = nc.dram_tensor("mask_t", (S, S), bf16, kind="ExternalInput")
    wq = nc.dram_tensor("wq", (P, TDIN, DG), bf16, kind="ExternalInput")
    wk = nc.dram_tensor("wk", (P, TDIN, DG), bf16, kind="ExternalInput")
    wv = nc.dram_tensor("wv", (P, TDIN, DG), bf16, kind="ExternalInput")
    # wo pre-arranged host-side to [64, HPG, D] (j, h, n) so each head's
    # 64 rows sit on partitions 0..63.
    wo = nc.dram_tensor("wo", (HD, HPG, D), bf16, kind="ExternalInput")
    qb = nc.dram_tensor("qb", (1, DG), bf16, kind="ExternalInput")
    kb = nc.dram_tensor("kb", (1, DG), bf16, kind="ExternalInput")
    vb = nc.dram_tensor("vb", (1, DG), bf16, kind="ExternalInput")
    out = nc.dram_tensor("out", (S, D), bf16, kind="ExternalOutput")

    with tile.TileContext(nc) as tc, ExitStack() as ctx:
        wpool = ctx.enter_context(tc.tile_pool(name="wpool", bufs=1))
        cpool = ctx.enter_context(tc.tile_pool(name="cpool", bufs=1))
        qkpool = ctx.enter_context(tc.tile_pool(name="qk", bufs=1))
        vpool = ctx.enter_context(tc.tile_pool(name="vsb", bufs=1))
        mpool = ctx.enter_context(tc.tile_pool(name="msk", bufs=1))
        ppool = ctx.enter_context(tc.tile_pool(name="ptile", bufs=3))
        spool = ctx.enter_context(tc.tile_pool(name="small", bufs=2))
        dpool = ctx.enter_context(tc.tile_pool(name="dscr", bufs=2, space="DRAM"))
        bigps = ctx.enter_context(tc.tile_pool(name="bigps", bufs=3, space="PSUM"))
        ops_pool = ctx.enter_context(tc.tile_pool(name="ops", bufs=1, space="PSUM"))

        # ---- persistent SBUF tensors -------------------------------------
        wq_sb = wpool.tile([P, TDIN, DG], bf16)
        wk_sb = wpool.tile([P, TDIN, DG], bf16)
        wv_sb = wpool.tile([P, TDIN, DG], bf16)
        for wsb_, wdr_ in ((wq_sb, wq), (wk_sb, wk), (wv_sb, wv)):
            nc.sync.dma_start(wsb_[:], wdr_.ap())
        wo_sb = wpool.tile([HD, HPG, D], bf16)
        nc.sync.dma_start(wo_sb[:], wo.ap())
        qb_sb = cpool.tile([1, DG], bf16)
        kb_sb = cpool.tile([1, DG], bf16)
        vb_sb = cpool.tile([1, DG], bf16)
        nc.sync.dma_start(qb_sb[:], qb.ap())
        nc.sync.dma_start(kb_sb[:], kb.ap())
        nc.sync.dma_start(vb_sb[:], vb.ap())
        # ones: row 0 used as [1, CW] rhs / [1, P] lhsT at partition 0;
        # row 64 used as [1, HD] lhsT at partition 64 (denominator bcast).
        ones2 = cpool.tile([1, CW], bf16)
        nc.vector.memset(ones2[:], 1.0)

        qT_sb = qkpool.tile([P, NT2, S], bf16)   # q projection, transposed
        kT_sb = qkpool.tile([P, NT2, S], bf16)
        # v blocks: per k-tile, per head: [v(64) | ones] -> 65 cols
        v_sb = vpool.tile([P, KT, HPG * (HD + 1)], bf16)
        nc.vector.memset(
            v_sb[:].rearrange("p s (h x) -> p s h x", h=HPG)[:, :, :, HD : HD + 1],
            1.0,
        )
        # ---- projections -------------------------------------------------
        # q, k: out qT[dout, s] = wq^T(stationary) x q^T(moving) + bias
        xpool_cm = tc.tile_pool(name="xin", bufs=2)
        xpool = xpool_cm.__enter__()
        NCH = S // 1024
        for xdram, wsb, bias_sb, dest in (
            (xq, wq_sb, qb_sb, qT_sb),
            (xk, wk_sb, kb_sb, kT_sb),
        ):
            for sc in range(NCH):
                xch = xpool.tile([P, TDIN, 1024], bf16, tag="xch")
                nc.sync.dma_start(
                    xch[:], xdram.ap()[:, :, sc * 1024 : (sc + 1) * 1024]
                )
                for half in range(2):
                    s0 = sc * 1024 + half * 512
                    for dt in range(NT2):
                        ps = bigps.tile(
                            [P, CW], f32, tag="ps", name=f"pj_{sc}_{half}_{dt}"
                        )
                        for ktl in range(TDIN):
                            nc.tensor.matmul(
                                ps[:],
                                lhsT=wsb[:, ktl, dt * P : (dt + 1) * P],
                                rhs=xch[:, ktl, half * 512 : (half + 1) * 512],
                                start=(ktl == 0),
                                stop=False,
                            )
                        nc.tensor.matmul(
                            ps[:],
                            lhsT=bias_sb[0:1, dt * P : (dt + 1) * P],
                            rhs=ones2[0:1, :],
                            start=False,
                            stop=True,
                        )
                        nc.scalar.copy(dest[:, dt, s0 : s0 + 512], ps[:])
        # v: natural layout [s, dout] + bias, drained per-head with ones col
        for sc in range(NCH):
            xch = xpool.tile([P, TDIN, 1024], bf16, tag="xch")
            nc.sync.dma_start(
                xch[:], xv.ap()[:, :, sc * 1024 : (sc + 1) * 1024]
            )
            for m in range(1024 // P):
                st = sc * (1024 // P) + m
                ps = bigps.tile([P, DG], f32, tag="ps", name=f"pv_{sc}_{m}")
                for ktl in range(TDIN):
                    nc.tensor.matmul(
                        ps[:],
                        lhsT=xch[:, ktl, m * P : (m + 1) * P],
                        rhs=wv_sb[:, ktl, :],
                        start=(ktl == 0),
                        stop=False,
                    )
                nc.tensor.matmul(
                    ps[:],
                    lhsT=ones2[0:1, 0:P],
                    rhs=vb_sb[:],
                    start=False,
                    stop=True,
                )
                nc.scalar.copy(
                    v_sb[:, st, :].rearrange("p (h x) -> p h x", h=HPG)[
                        :, :, 0:HD
                    ],
                    ps[:].rearrange("p (h x) -> p h x", h=HPG),
                )
        xpool_cm.__exit__(None, None, None)

        # mask load issued after projection DMAs so it doesn't hog queues
        mask_sb = mpool.tile([P, KT, S], bf16)
        mk_r = mk.ap().rearrange("(t p) q -> p t q", p=P)
        for kt in range(KT):
            nc.sync.dma_start(mask_sb[:, kt, :], mk_r[:, kt, :])

        # ---- attention ---------------------------------------------------
        # One head at a time; score psum triple-buffered so the PE can run
        # up to 3 k-tiles ahead of the exp/mask/PV chain.
        opool_sb = ctx.enter_context(tc.tile_pool(name="otn", bufs=1))
        otn_sb = opool_sb.tile([HD, HPG, S], bf16)

        def emit_outproj(st):
            osb2 = ppool.tile([P, D], bf16, tag="outsb", name=f"outsb_{st}")
            for nch in range(2):
                op_ps = bigps.tile(
                    [P, 512], f32, tag="ps", name=f"ops2_{st}_{nch}"
                )
                for h_ in range(HPG):
                    nc.tensor.matmul(
                        op_ps[:],
                        lhsT=otn_sb[:, h_, st * P : (st + 1) * P],
                        rhs=wo_sb[:, h_, nch * 512 : (nch + 1) * 512],
                        start=(h_ == 0),
                        stop=(h_ == HPG - 1),
                    )
                nc.vector.tensor_copy(
                    osb2[:, nch * 512 : (nch + 1) * 512], op_ps[:]
                )
            nc.sync.dma_start(out.ap()[st * P : (st + 1) * P, :], osb2[:])

        pending_st = []
        for qc in range(QC):
            for h in range(HPG):
                t, po = h // 2, (h % 2) * HD
                o_ps = ops_pool.tile(
                    [HD + 1, QW], f32, tag="ops", name=f"ops_{qc}_{h}"
                )
                for kt in range(KT):
                    s_ps = bigps.tile(
                        [P, QW], f32, tag="ps", name=f"sps_{qc}_{h}_{kt}"
                    )
                    for hf in range(2):
                        nc.tensor.matmul(
                            s_ps[:, hf * 512 : (hf + 1) * 512],
                            lhsT=kT_sb[po : po + HD, t, kt * P : (kt + 1) * P],
                            rhs=qT_sb[
                                po : po + HD,
                                t,
                                qc * QW + hf * 512 : qc * QW + (hf + 1) * 512,
                            ],
                            start=True,
                            stop=True,
                        )
                    pt = ppool.tile(
                        [P, QW], bf16, tag="p", name=f"pt_{qc}_{h}_{kt}"
                    )
                    nc.scalar.activation(pt[:], s_ps[:], EXP, scale=0.125)
                    nc.vector.tensor_mul(
                        pt[:], pt[:], mask_sb[:, kt, qc * QW : (qc + 1) * QW]
                    )
                    for hf in range(2):
                        nc.tensor.matmul(
                            o_ps[:, hf * 512 : (hf + 1) * 512],
                            lhsT=v_sb[:, kt, h * 65 : (h + 1) * 65],
                            rhs=pt[:, hf * 512 : (hf + 1) * 512],
                            start=(kt == 0),
                            stop=(kt == KT - 1),
                        )
                # softmax normalization (no PE): approx-recip of the
                # denominator row, DRAM-bounce broadcast, one TT multiply.
                rec65 = spool.tile([HD + 1, QW], f32, tag="rec")
                nc.vector.reciprocal_approx_fast(out=rec65[:], in_=o_ps[:])
                osb = spool.tile([HD, QW], f32, tag="osb")
                nc.vector.tensor_copy(osb[:], o_ps[0:HD, :])
                scr = dpool.tile([1, QW], f32, tag="scr", name=f"scr_{qc}_{h}")
                nc.sync.dma_start(scr[:], rec65[HD : HD + 1, :])
                rb = spool.tile([HD, QW], f32, tag="rb")
                nc.sync.dma_start(rb[:], scr[:].to_broadcast((HD, QW)))
                nc.vector.tensor_mul(
                    otn_sb[:, h, qc * QW : (qc + 1) * QW], osb[:], rb[:]
                )
                for _ in range(2):
                    if pending_st:
                        emit_outproj(pending_st.pop(0))

            pending_st.extend(range(qc * (QW // P), (qc + 1) * (QW // P)))

        for st in pending_st:
            emit_outproj(st)

    nc.compile()
    return nc


@functools.lru_cache(maxsize=1)
def _graph():
    return build_graph()


def make_in_maps(
    query, key, value, mask,
    wq_kernel, wq_bias, wk_kernel, wk_bias,
    wv_kernel, wv_bias, wo_kernel, wo_bias,
):
    q = np.asarray(query, np.float32)
    k = np.asarray(key, np.float32)
    v = np.asarray(value, np.float32)
    mask = np.asarray(mask)
    wqk = np.asarray(wq_kernel, np.float32)
    wkk = np.asarray(wk_kernel, np.float32)
    wvk = np.asarray(wv_kernel, np.float32)
    wok = np.asarray(wo_kernel, np.float32)

    def tile_x(a):  # [S, D] -> [P, TDIN, S] pre-tiled transpose
        return np.ascontiguousarray(
            a.T.reshape(TDIN, P, S).transpose(1, 0, 2)
        ).astype(BF16)

    xt = [[tile_x(x[b]) for x in (q, k, v)] for b in range(B)]
    mt = [
        np.ascontiguousarray(mask[b].T.astype(np.float32)).astype(BF16)
        for b in range(B)
    ]
    in_maps = []
    for c in range(NCORES):
        b, g = divmod(c, GH)
        cs = slice(g * DG, (g + 1) * DG)
        wo_arr = np.ascontiguousarray(
            wok[cs, :].reshape(HPG, HD, D).transpose(1, 0, 2)
        ).astype(BF16)
        in_maps.append(
            {
                "xq_t": xt[b][0],
                "xk_t": xt[b][1],
                "xv_t": xt[b][2],
                "mask_t": mt[b],
                "wq": np.ascontiguousarray(wqk[:, cs].reshape(TDIN, P, DG).transpose(1, 0, 2)).astype(BF16),
                "wk": np.ascontiguousarray(wkk[:, cs].reshape(TDIN, P, DG).transpose(1, 0, 2)).astype(BF16),
                "wv": np.ascontiguousarray(wvk[:, cs].reshape(TDIN, P, DG).transpose(1, 0, 2)).astype(BF16),
                "wo": wo_arr,
                "qb": np.asarray(wq_bias, np.float32)[cs].reshape(1, DG).astype(BF16),
                "kb": np.asarray(wk_bias, np.float32)[cs].reshape(1, DG).astype(BF16),
                "vb": np.asarray(wv_bias, np.float32)[cs].reshape(1, DG).astype(BF16),
            }
        )
    return in_maps


def combine_outputs(results, wo_bias):
    outs = np.stack([np.asarray(r["out"], np.float32) for r in results])
    full = outs.reshape(B, GH, S, D).sum(axis=1)
    return (full + np.asarray(wo_bias, np.float32)[None, None, :]).astype(
        np.float32
    )


def kernel(**inputs):
    from concourse import bass_utils

    nc = _graph()
    in_maps = make_in_maps(**inputs)
    res = bass_utils.run_bass_kernel_spmd(
        nc, in_maps, core_ids=list(range(NCORES))
    )
    return combine_outputs(res.results, inputs["wo_bias"])


# revision 31
# speedup vs baseline: 1.4656x; 1.2151x over previous
# Multi-head attention (B=2, S=2048, D=1024, H=16) on 8 TRN2 NeuronCores.
#
# Sharding (hardcoded): core c in [0..8) handles batch b = c//4 and head
# group g = c%4 (4 heads = 256 output features of wq/wk/wv, 256 input rows
# of wo). Each core computes a partial output projection [S, D]; the host
# sums the 4 partials per batch and adds wo_bias (row-parallel unshard).
#
# Device-side layout choices:
#   - activations enter transposed ([D, S]) so every matmul contracts over
#     the partition axis with no on-device transposes;
#   - scores are computed transposed (S^T[k, q]) so softmax(P) feeds the
#     P@V matmul directly (contraction over k on partitions);
#   - the softmax denominator comes free as an extra ones-column appended
#     to each head's V block (output row 64 of the PV accumulation);
#   - matmuls run in float32r (full-rate fp32 path for moving dim >= 256);
#     P/V/out-proj run in bf16.
import functools
import sys

import numpy as np

try:
    import concourse  # noqa: F401
except ImportError:  # harness env without the default path
    sys.path.insert(0, "/opt/trn_rl_repo")
    sys.path.insert(0, "/opt/pypackages")

import ml_dtypes

BF16 = ml_dtypes.bfloat16

B, S, D, H = 2, 2048, 1024, 16
HD = D // H          # 64
NCORES = 8
GH = 4               # head groups (tensor-parallel)
HPG = H // GH        # heads per group = 4
DG = D // GH         # features per group = 256
P = 128              # partitions
TDIN = D // P        # 8 din tiles
SC = 4               # s-chunks of 512 for projections
CW = S // SC         # 512
QC = 2               # q-chunks of 1024 for attention
QW = S // QC         # 1024
KT = S // P          # 16 k tiles
NT2 = DG // P        # 2 dout tiles per group


def build_graph():
    """Build the SPMD Bass graph (identical on all 8 cores)."""
    from contextlib import ExitStack

    from concourse import bacc, mybir, tile

    f32 = mybir.dt.float32
    f32r = mybir.dt.float32r
    bf16 = mybir.dt.bfloat16
    EXP = mybir.ActivationFunctionType.Exp

    nc = bacc.Bacc(
        "TRN2", target_bir_lowering=False, debug=False, num_devices=NCORES
    )

    xq = nc.dram_tensor("xq_t", (P, TDIN, S), bf16, kind="ExternalInput")
    xk = nc.dram_tensor("xk_t", (P, TDIN, S), bf16, kind="ExternalInput")
    xv = nc.dram_tensor("xv_t", (P, TDIN, S), bf16, kind="ExternalInput")
    mk = nc.dram_tensor("mask_t", (S, S), bf16, kind="ExternalInput")
    wq = nc.dram_tensor("wq", (P, TDIN, DG), bf16, kind="ExternalInput")
    wk = nc.dram_tensor("wk", (P, TDIN, DG), bf16, kind="ExternalInput")
    wv = nc.dram_tensor("wv", (P, TDIN, DG), bf16, kind="ExternalInput")
    # wo pre-arranged host-side to [64, HPG, D] (j, h, n) so each head's
    # 64 rows sit on partitions 0..63.
    wo = nc.dram_tensor("wo", (HD, HPG, D), bf16, kind="ExternalInput")
    qb = nc.dram_tensor("qb", (1, DG), bf16, kind="ExternalInput")
    kb = nc.dram_tensor("kb", (1, DG), bf16, kind="ExternalInput")
    vb = nc.dram_tensor("vb", (1, DG), bf16, kind="ExternalInput")
    out = nc.dram_tensor("out", (S, D), bf16, kind="ExternalOutput")

    with tile.TileContext(nc) as tc, ExitStack() as ctx:
        wpool = ctx.enter_context(tc.tile_pool(name="wpool", bufs=1))
        cpool = ctx.enter_context(tc.tile_pool(name="cpool", bufs=1))
        qkpool = ctx.enter_context(tc.tile_pool(name="qk", bufs=1))
        vpool = ctx.enter_context(tc.tile_pool(name="vsb", bufs=1))
        mpool = ctx.enter_context(tc.tile_pool(name="msk", bufs=1))
        ppool = ctx.enter_context(tc.tile_pool(name="ptile", bufs=3))
        spool = ctx.enter_context(tc.tile_pool(name="small", bufs=2))
        dpool = ctx.enter_context(tc.tile_pool(name="dscr", bufs=2, space="DRAM"))
        bigps = ctx.enter_context(tc.tile_pool(name="bigps", bufs=3, space="PSUM"))
        ops_pool = ctx.enter_context(tc.tile_pool(name="ops", bufs=1, space="PSUM"))

        # ---- persistent SBUF tensors -------------------------------------
        wq_sb = wpool.tile([P, TDIN, DG], bf16)
        wk_sb = wpool.tile([P, TDIN, DG], bf16)
        wv_sb = wpool.tile([P, TDIN, DG], bf16)
        for wsb_, wdr_ in ((wq_sb, wq), (wk_sb, wk), (wv_sb, wv)):
            nc.sync.dma_start(wsb_[:], wdr_.ap())
        wo_sb = wpool.tile([HD, HPG, D], bf16)
        nc.sync.dma_start(wo_sb[:], wo.ap())
        qb_sb = cpool.tile([1, DG], bf16)
        kb_sb = cpool.tile([1, DG], bf16)
        vb_sb = cpool.tile([1, DG], bf16)
        nc.sync.dma_start(qb_sb[:], qb.ap())
        nc.sync.dma_start(kb_sb[:], kb.ap())
        nc.sync.dma_start(vb_sb[:], vb.ap())
        # ones: row 0 used as [1, CW] rhs / [1, P] lhsT at partition 0;
        # row 64 used as [1, HD] lhsT at partition 64 (denominator bcast).
        ones2 = cpool.tile([1, CW], bf16)
        nc.vector.memset(ones2[:], 1.0)

        qT_sb = qkpool.tile([P, NT2, S], bf16)   # q projection, transposed
        kT_sb = qkpool.tile([P, NT2, S], bf16)
        # v blocks: per k-tile, per head: [v(64) | ones] -> 65 cols
        v_sb = vpool.tile([P, KT, HPG * (HD + 1)], bf16)
        nc.vector.memset(
            v_sb[:].rearrange("p s (h x) -> p s h x", h=HPG)[:, :, :, HD : HD + 1],
            1.0,
        )
        # ---- projections -------------------------------------------------
        # q, k: out qT[dout, s] = wq^T(stationary) x q^T(moving) + bias
        xpool_cm = tc.tile_pool(name="xin", bufs=2)
        xpool = xpool_cm.__enter__()
        NCH = S // 1024
        for xdram, wsb, bias_sb, dest in (
            (xq, wq_sb, qb_sb, qT_sb),
            (xk, wk_sb, kb_sb, kT_sb),
        ):
            for sc in range(NCH):
                xch = xpool.tile([P, TDIN, 1024], bf16, tag="xch")
                nc.sync.dma_start(
                    xch[:], xdram.ap()[:, :, sc * 1024 : (sc + 1) * 1024]
                )
                for half in range(2):
                    s0 = sc * 1024 + half * 512
                    for dt in range(NT2):
                        ps = bigps.tile(
                            [P, CW], f32, tag="ps", name=f"pj_{sc}_{half}_{dt}"
                        )
                        for ktl in range(TDIN):
                            nc.tensor.matmul(
                                ps[:],
                                lhsT=wsb[:, ktl, dt * P : (dt + 1) * P],
                                rhs=xch[:, ktl, half * 512 : (half + 1) * 512],
                                start=(ktl == 0),
                                stop=False,
                            )
                        nc.tensor.matmul(
                            ps[:],
                            lhsT=bias_sb[0:1, dt * P : (dt + 1) * P],
                            rhs=ones2[0:1, :],
                            start=False,
                            stop=True,
                        )
                        nc.scalar.copy(dest[:, dt, s0 : s0 + 512], ps[:])
        # v: natural layout [s, dout] + bias, drained per-head with ones col
        for sc in range(NCH):
            xch = xpool.tile([P, TDIN, 1024], bf16, tag="xch")
            nc.sync.dma_start(
                xch[:], xv.ap()[:, :, sc * 1024 : (sc + 1) * 1024]
            )
            for m in range(1024 // P):
                st = sc * (1024 // P) + m
                ps = bigps.tile([P, DG], f32, tag="ps", name=f"pv_{sc}_{m}")
                for ktl in range(TDIN):
                    nc.tensor.matmul(
                        ps[:],
                        lhsT=xch[:, ktl, m * P : (m + 1) * P],
                        rhs=wv_sb[:, ktl, :],
                        start=(ktl == 0),
                        stop=False,
                    )
                nc.tensor.matmul(
                    ps[:],
                    lhsT=ones2[0:1, 0:P],
                    rhs=vb_sb[:],
                    start=False,
                    stop=True,
                )
                nc.scalar.copy(
                    v_sb[:, st, :].rearrange("p (h x) -> p h x", h=HPG)[
                        :, :, 0:HD
                    ],
                    ps[:].rearrange("p (h x) -> p h x", h=HPG),
                )
        xpool_cm.__exit__(None, None, None)

        # mask load issued after projection DMAs so it doesn't hog queues
        mask_sb = mpool.tile([P, KT, S], bf16)
        mk_r = mk.ap().rearrange("(t p) q -> p t q", p=P)
        for kt in range(KT):
            nc.sync.dma_start(mask_sb[:, kt, :], mk_r[:, kt, :])

        # ---- attention ---------------------------------------------------
        # One head at a time; score psum triple-buffered so the PE can run
        # up to 3 k-tiles ahead of the exp/mask/PV chain.
        opool_sb = ctx.enter_context(tc.tile_pool(name="otn", bufs=1))
        otn_sb = opool_sb.tile([HD, HPG, S], bf16)

        def emit_outproj(st):
            osb2 = ppool.tile([P, D], bf16, tag="outsb", name=f"outsb_{st}")
            for nch in range(2):
                op_ps = bigps.tile(
                    [P, 512], f32, tag="ps", name=f"ops2_{st}_{nch}"
                )
                for h_ in range(HPG):
                    nc.tensor.matmul(
                        op_ps[:],
                        lhsT=otn_sb[:, h_, st * P : (st + 1) * P],
                        rhs=wo_sb[:, h_, nch * 512 : (nch + 1) * 512],
                        start=(h_ == 0),
                        stop=(h_ == HPG - 1),
                    )
                nc.vector.tensor_copy(
                    osb2[:, nch * 512 : (nch + 1) * 512], op_ps[:]
                )
            nc.sync.dma_start(out.ap()[st * P : (st + 1) * P, :], osb2[:])

        pending_st = []
        for qc in range(QC):
            for h in range(HPG):
                t, po = h // 2, (h % 2) * HD
                o_ps = ops_pool.tile(
                    [HD + 1, QW], f32, tag="ops", name=f"ops_{qc}_{h}"
                )
                for kt in range(KT):
                    s_ps = bigps.tile(
                        [P, QW], f32, tag="ps", name=f"sps_{qc}_{h}_{kt}"
                    )
                    for hf in range(2):
                        nc.tensor.matmul(
                            s_ps[:, hf * 512 : (hf + 1) * 512],
                            lhsT=kT_sb[po : po + HD, t, kt * P : (kt + 1) * P],
                            rhs=qT_sb[
                                po : po + HD,
                                t,
                                qc * QW + hf * 512 : qc * QW + (hf + 1) * 512,
                            ],
                            start=True,
                            stop=True,
                        )
                    pt = ppool.tile(
                        [P, QW], bf16, tag="p", name=f"pt_{qc}_{h}_{kt}"
                    )
                    nc.scalar.activation(pt[:], s_ps[:], EXP, scale=0.125)
                    nc.vector.tensor_mul(
                        pt[:], pt[:], mask_sb[:, kt, qc * QW : (qc + 1) * QW]
                    )
                    for hf in range(2):
                        nc.tensor.matmul(
                            o_ps[:, hf * 512 : (hf + 1) * 512],
                            lhsT=v_sb[:, kt, h * 65 : (h + 1) * 65],
                            rhs=pt[:, hf * 512 : (hf + 1) * 512],
                            start=(kt == 0),
                            stop=(kt == KT - 1),
                        )
                # softmax normalization (no PE): approx-recip of the
                # denominator row, DRAM-bounce broadcast, one TT multiply.
                rec65 = spool.tile([HD + 1, QW], f32, tag="rec")
                nc.vector.reciprocal_approx_fast(out=rec65[:], in_=o_ps[:])
                osb = spool.tile([HD, QW], f32, tag="osb")
                nc.vector.tensor_copy(osb[:], o_ps[0:HD, :])
                scr = dpool.tile([1, QW], f32, tag="scr", name=f"scr_{qc}_{h}")
                nc.sync.dma_start(scr[:], rec65[HD : HD + 1, :])
                rb = spool.tile([HD, QW], f32, tag="rb")
                nc.sync.dma_start(rb[:], scr[:].to_broadcast((HD, QW)))
                nc.vector.tensor_mul(
                    otn_sb[:, h, qc * QW : (qc + 1) * QW], osb[:], rb[:]
                )

            pending_st.extend(range(qc * (QW // P), (qc + 1) * (QW // P)))

        for st in pending_st:
            emit_outproj(st)

    nc.compile()
    return nc


@functools.lru_cache(maxsize=1)
def _graph():
    return build_graph()


def make_in_maps(
    query, key, value, mask,
    wq_kernel, wq_bias, wk_kernel, wk_bias,
    wv_kernel, wv_bias, wo_kernel, wo_bias,
):
    q = np.asarray(query, np.float32)
    k = np.asarray(key, np.float32)
    v = np.asarray(value, np.float32)
    mask = np.asarray(mask)
    wqk = np.asarray(wq_kernel, np.float32)
    wkk = np.asarray(wk_kernel, np.float32)
    wvk = np.asarray(wv_kernel, np.float32)
    wok = np.asarray(wo_kernel, np.float32)

    def tile_x(a):  # [S, D] -> [P, TDIN, S] pre-tiled transpose
        return np.ascontiguousarray(
            a.T.reshape(TDIN, P, S).transpose(1, 0, 2)
        ).astype(BF16)

    xt = [[tile_x(x[b]) for x in (q, k, v)] for b in range(B)]
    mt = [
        np.ascontiguousarray(mask[b].T.astype(np.float32)).astype(BF16)
        for b in range(B)
    ]
    in_maps = []
    for c in range(NCORES):
        b, g = divmod(c, GH)
        cs = slice(g * DG, (g + 1) * DG)
        wo_arr = np.ascontiguousarray(
            wok[cs, :].reshape(HPG, HD, D).transpose(1, 0, 2)
        ).astype(BF16)
        in_maps.append(
            {
                "xq_t": xt[b][0],
                "xk_t": xt[b][1],
                "xv_t": xt[b][2],
                "mask_t": mt[b],
                "wq": np.ascontiguousarray(wqk[:, cs].reshape(TDIN, P, DG).transpose(1, 0, 2)).astype(BF16),
                "wk": np.ascontiguousarray(wkk[:, cs].reshape(TDIN, P, DG).transpose(1, 0, 2)).astype(BF16),
                "wv": np.ascontiguousarray(wvk[:, cs].reshape(TDIN, P, DG).transpose(1, 0, 2)).astype(BF16),
                "wo": wo_arr,
                "qb": np.asarray(wq_bias, np.float32)[cs].reshape(1, DG).astype(BF16),
                "kb": np.asarray(wk_bias, np.float32)[cs].reshape(1, DG).astype(BF16),
                "vb": np.asarray(wv_bias, np.float32)[cs].reshape(1, DG).astype(BF16),
            }
        )
    return in_maps


def combine_outputs(results, wo_bias):
    outs = np.stack([np.asarray(r["out"], np.float32) for r in results])
    full = outs.reshape(B, GH, S, D).sum(axis=1)
    return (full + np.asarray(wo_bias, np.float32)[None, None, :]).astype(
        np.float32
    )


def kernel(**inputs):
    from concourse import bass_utils

    nc = _graph()
    in_maps = make_in_maps(**inputs)
    res = bass_utils.run_bass_kernel_spmd(
        nc, in_maps, core_ids=list(range(NCORES))
    )
    return combine_outputs(res.results, inputs["wo_bias"])
